# revision 34
# baseline (speedup 1.0000x reference)
"""Causal self-attention with RoPE on 8 Trainium2 NeuronCores.

Sharding: batch (4) x head-group (2 groups of 8 heads) -> 8 cores.
Schedule: pair-level software pipeline.  The attention phase for head
pair p is ACT(exp)-bound (~1147ns/unit vs ~640ns PE/unit), so the PE
slack inside pair p's attention is filled with the V/QK projections +
RoPE of pair p+1, and with the output projection during the last pair.
Scores matmuls are emitted h0/h1-adjacent so the K=64 row-tiled pairs
run concurrently on disjoint PE row groups.  Inputs are host-reordered
so every DMA has fat (4-16KB) per-partition lines, split across three
engine queues; junk matmuls on tri warm the PE HAM clock during the
initial DMA latency.  Output is fp16 (summed per-batch on host).
"""
import sys

sys.path.insert(0, "/opt/trn_rl_repo")

import numpy as np

import concourse.bass as bass  # noqa: F401
import concourse.mybir as mybir
import concourse.tile as tile
from concourse import bacc
from concourse.bass_utils import run_bass_kernel_spmd

dt = mybir.dt
F32, F16 = dt.float32, dt.float16
ALU = mybir.AluOpType
EXP = mybir.ActivationFunctionType.Exp

ROPE_BASE = 10000.0
# swap adjacent partition pairs within each 32-partition quadrant
SHUF_MASK = [i ^ 1 for i in range(32)]
# row r of a 64-feature head holds feature PERM64[r] (pairs (k, k+32)
# interleaved so rotate_half becomes the adjacent-pair swap above)
PERM64 = [(r // 2) if r % 2 == 0 else (r // 2 + 32) for r in range(64)]


def build_core_program(S=2048, D=1024, HL=8, hd=64):
    assert hd == 64
    NP = HL // 2           # 4 head pairs
    DT = D // 128          # 8 contraction tiles
    SC = S // 512          # 4 sequence chunks
    ST = S // 128          # 16 seq tiles
    IC = S // 256          # 8 attention query chunks per pair
    NDC = D // 512         # 2 out-proj column chunks
    scale = hd ** -0.5

    nc = bacc.Bacc("TRN2", target_bir_lowering=False, debug=False)
    # host-reordered fat-line layouts
    x_d = nc.dram_tensor("xh", [128, SC, DT, 512], F16, kind="ExternalInput").ap()
    Wqk_d = nc.dram_tensor("Wqk", [128, DT, NP * 256], F16, kind="ExternalInput").ap()
    Wv_d = nc.dram_tensor("Wv", [128, DT, HL * hd], F16, kind="ExternalInput").ap()
    Wout_d = nc.dram_tensor("Wout", [128, NP, NDC, 512], F16, kind="ExternalInput").ap()
    cos_d = nc.dram_tensor("cosT", [64, S], F16, kind="ExternalInput").ap()
    sin_d = nc.dram_tensor("sinT", [64, S], F16, kind="ExternalInput").ap()
    tri_d = nc.dram_tensor("tri", [128, 128], F16, kind="ExternalInput").ap()
    rotP_d = nc.dram_tensor("rotP", [128, 128], F16, kind="ExternalInput").ap()
    y_d = nc.dram_tensor("y", [S, D], F16, kind="ExternalOutput").ap()

    with tile.TileContext(nc) as tc:
        with tc.tile_pool(name="persist", bufs=1) as pp, \
             tc.tile_pool(name="q16p", bufs=4) as q16p, \
             tc.tile_pool(name="expp", bufs=6) as expp, \
             tc.tile_pool(name="normp", bufs=4) as normp, \
             tc.tile_pool(name="ystp", bufs=2) as ystp, \
             tc.tile_pool(name="projps", bufs=2, space="PSUM") as projps, \
             tc.tile_pool(name="sps", bufs=2, space="PSUM") as sps, \
             tc.tile_pool(name="pavp", bufs=2, space="PSUM") as pavp:

            # ---------------- persistent SBUF tensors ----------------
            xq = pp.tile([128, SC, DT, 512], F16, tag="xq")
            wv = pp.tile([128, DT, HL * hd], F16, tag="wv")
            wqk = pp.tile([128, DT, NP * 256], F16, tag="wqk")
            wout = pp.tile([128, NP, NDC, 512], F16, tag="wout")
            qkT = [pp.tile([128, S], F16, tag=f"qkT{j}", name=f"qkT{j}")
                   for j in range(2 * NP)]
            v_sb = pp.tile([128, ST, HL, 66], F16, tag="v_sb")
            outT = [pp.tile([128, S], F16, tag=f"outT{p}", name=f"outT{p}")
                    for p in range(NP)]
            cosT = pp.tile([128, S], F16, tag="cosT")
            sinT = pp.tile([128, S], F16, tag="sinT")
            tri = pp.tile([128, 128], F16, tag="tri")
            rotP = pp.tile([128, 128], F16, tag="rotP")

            # DMAs: single proven sync queue; ordered by first consumer.
            nc.sync.dma_start(out=tri[:], in_=tri_d[:])
            nc.sync.dma_start(out=rotP[:], in_=rotP_d[:])
            nc.sync.dma_start(out=wv[:], in_=Wv_d[:])
            nc.sync.dma_start(out=xq[:, 0, :, :], in_=x_d[:, 0, :, :])
            nc.sync.dma_start(out=wqk[:], in_=Wqk_d[:])
            nc.sync.dma_start(out=cosT[0:64, :], in_=cos_d[:])
            nc.sync.dma_start(out=sinT[0:64, :], in_=sin_d[:])
            nc.vector.tensor_copy(cosT[64:128, :], cosT[0:64, :])
            nc.vector.tensor_copy(sinT[64:128, :], sinT[0:64, :])
            for sc in range(1, SC):
                nc.sync.dma_start(out=xq[:, sc, :, :], in_=x_d[:, sc, :, :])
            nc.sync.dma_start(out=wout[:], in_=Wout_d[:])
            nc.vector.memset(v_sb[:, :, :, 64:65], 1.0)

            # PE warmup on tri during initial DMA latency (HAM + P-state)
            junk = sps.tile([128, 2, 2, 256], F32, tag="spt", name="junk")
            for _ in range(46):
                nc.tensor.matmul(junk[:, 0, 0, 0:128], tri[:], tri[:],
                                 start=True, stop=True)


            # ---------------- filler units (PE work for pipelining) --------
            def v_unit(p, st):
                sc, r = st // 4, st % 4
                vps = projps.tile([128, 128], F32, tag="pjps", name="vps")
                for ddt in range(DT):
                    nc.tensor.matmul(
                        vps[:],
                        xq[:, sc, ddt, r * 128:(r + 1) * 128],
                        wv[:, ddt, p * 128:(p + 1) * 128],
                        start=(ddt == 0), stop=(ddt == DT - 1))
                nc.vector.tensor_copy(
                    v_sb[:, st, 2 * p:2 * p + 2, 0:64],
                    vps[:].rearrange("p (h c) -> p h c", h=2))

            def qk_half_a(p, jh, sc, st8):
                qkps = projps.tile([128, 512], F32, tag="pjps", name="qkps")
                st8["qkps"] = qkps
                for ddt in range(DT // 2):
                    nc.tensor.matmul(
                        qkps[:],
                        wqk[:, ddt, p * 256 + jh * 128:p * 256 + jh * 128 + 128],
                        xq[:, sc, ddt, :],
                        start=(ddt == 0), stop=False)

            def qk_half_b(p, jh, sc, st8):
                jt = 2 * p + jh
                ss = slice(sc * 512, (sc + 1) * 512)
                qkps = st8["qkps"]
                for ddt in range(DT // 2, DT):
                    nc.tensor.matmul(
                        qkps[:],
                        wqk[:, ddt, p * 256 + jh * 128:p * 256 + jh * 128 + 128],
                        xq[:, sc, ddt, :],
                        start=False, stop=(ddt == DT - 1))
                q16 = q16p.tile([128, 512], F16, tag="q16", name="q16")
                nc.vector.tensor_copy(q16[:], qkps[:])
                rotm = q16p.tile([128, 512], F16, tag="q16", name="rotm")
                nc.vector.stream_shuffle(rotm[:], q16[:], SHUF_MASK)
                nc.vector.tensor_tensor(rotm[:], rotm[:], sinT[:, ss],
                                        ALU.mult)
                nc.vector.tensor_tensor(qkT[jt][:, ss], q16[:], cosT[:, ss],
                                        ALU.mult)
                nc.vector.tensor_tensor(qkT[jt][:, ss], qkT[jt][:, ss],
                                        rotm[:], ALU.add)

            def qk_unit(p, jh, sc):
                st8 = {}
                qk_half_a(p, jh, sc, st8)
                qk_half_b(p, jh, sc, st8)

            def out_half_a(st, h8):
                yp2 = [projps.tile([128, 512], F32, tag="pjps", name="yps")
                       for _ in range(NDC)]
                h8["yp2"] = yp2
                for pb in range(NP // 2):
                    for dc in range(NDC):
                        nc.tensor.matmul(
                            yp2[dc][:],
                            outT[pb][:, st * 128:(st + 1) * 128],
                            wout[:, pb, dc, :],
                            start=(pb == 0), stop=False)

            def out_half_b(st, h8):
                yp2 = h8["yp2"]
                for pb in range(NP // 2, NP):
                    for dc in range(NDC):
                        nc.tensor.matmul(
                            yp2[dc][:],
                            outT[pb][:, st * 128:(st + 1) * 128],
                            wout[:, pb, dc, :],
                            start=False, stop=(pb == NP - 1))
                yst = ystp.tile([128, NDC * 512], F16, tag="yst", name="yst")
                for dc in range(NDC):
                    nc.vector.tensor_copy(
                        yst[:, dc * 512:(dc + 1) * 512], yp2[dc][:])
                nc.sync.dma_start(
                    out=y_d[st * 128:(st + 1) * 128, :], in_=yst[:])

            def out_unit(st):
                h8 = {}
                out_half_a(st, h8)
                out_half_b(st, h8)

            # ---------------- attention, software-pipelined ----------------
            pend = {"av": None, "norm": None}

            def flush_pend():
                if pend["av"] is not None:
                    pend["av"]()
                    pend["av"] = None
                if pend["norm"] is not None:
                    pend["norm"]()
                    pend["norm"] = None

            def attn_chunk(p, t, unit_cb):
                # unit_cb() is invoked once per key-unit, right after that
                # unit's PE attention work, to pump filler matmuls into the
                # PE stream while ACT chews on exp.
                qT, kT = qkT[2 * p], qkT[2 * p + 1]
                qs = slice(t * 256, (t + 1) * 256)
                pav = []
                njp = t + 1
                prev = None   # (jp, et) awaiting AV emission

                def emit_av(jp, et):
                    for jj in range(2):
                        jt = 2 * jp + jj
                        qlo = 128 if (jp == njp - 1 and jj == 1) else 0
                        for hh in range(2):
                            h = 2 * p + hh
                            nc.tensor.matmul(
                                pav[hh][0:65, qlo:256],
                                v_sb[:, jt, h, 0:65],
                                et[:, hh, jj, qlo:256],
                                start=(jp == 0 and jj == 0),
                                stop=(jp == njp - 1 and jj == 1))

                for jp in range(njp):
                    spt = sps.tile([128, 2, 2, 256], F32, tag="spt",
                                   name="spt")
                    diag = (jp == njp - 1)
                    for jj in range(2):
                        jt = 2 * jp + jj
                        for hh in range(2):
                            hb = 64 * hh
                            nc.tensor.matmul(
                                spt[:, hh, jj, :],
                                kT[hb:hb + 64, jt * 128:(jt + 1) * 128],
                                qT[hb:hb + 64, qs],
                                start=True, stop=True)
                    et = expp.tile([128, 2, 2, 256], F16, tag="expp",
                                   name="et")
                    nc.scalar.activation(et[:], spt[:], EXP, scale=scale)
                    if diag:
                        for hh in range(2):
                            for jj in range(2):
                                nc.vector.tensor_tensor(
                                    et[:, hh, jj, 128 * jj:128 * jj + 128],
                                    et[:, hh, jj, 128 * jj:128 * jj + 128],
                                    tri[:], ALU.mult)
                    if jp == 0:
                        flush_pend()
                        pav.extend(
                            pavp.tile([128, 256], F32, tag="pav", name="pav")
                            for _ in range(2))
                    else:
                        emit_av(*prev)
                    unit_cb()
                    prev = (jp, et)

                jp_l, et_l = prev
                pend["av"] = lambda: emit_av(jp_l, et_l)

                def emit_norm(pav=pav, p=p, qs=qs):
                    for hh in range(2):
                        srow = normp.tile([1, 256], F32, tag="srow",
                                          name="srow")
                        nc.vector.tensor_copy(srow[:], pav[hh][64:65, :])
                        rstage = normp.tile([1, 256], F32, tag="rst",
                                            name="rst")
                        nc.vector.reciprocal_approx_fast(
                            out=rstage[:], in_=srow[:])
                        brec = normp.tile([64, 256], F32, tag="brec",
                                          name="brec")
                        nc.gpsimd.partition_broadcast(brec[:], rstage[:])
                        nc.vector.tensor_tensor(
                            outT[p][64 * hh:64 * hh + 64, qs],
                            pav[hh][0:64, :], brec[:], ALU.mult)

                pend["norm"] = emit_norm

            # ---------------- schedule ----------------
            # Filler FIFO pumped one credit-slice per attention unit; gated
            # force-drains guarantee a pair's inputs exist before use.
            fill = []
            emitted = [0]
            added = [0]

            def pump(credit):
                while fill and credit > 0:
                    cost, fn = fill.pop(0)
                    fn()
                    emitted[0] += 1
                    credit -= cost

            def pump_until(n):
                while emitted[0] < n:
                    cost, fn = fill.pop(0)
                    fn()
                    emitted[0] += 1

            def addf(cost, fn):
                fill.append((cost, fn))
                added[0] += 1

            def add_v(p, st):
                addf(430, lambda: v_unit(p, st))

            def add_qk(p, jh, sc):
                st8 = {}
                addf(870, lambda: qk_half_a(p, jh, sc, st8))
                addf(870, lambda: qk_half_b(p, jh, sc, st8))

            # Prologue: just enough of pair 0 for its first two chunks.
            for st in range(4):
                v_unit(0, st)
            for jh in range(2):
                qk_unit(0, jh, 0)
            # Pair-0 leftovers, ordered so chunk gates pass; then pair 1.
            for jh in range(2):
                add_qk(0, jh, 1)
            for st in (4, 5, 6, 7):
                add_v(0, st)
            for jh in range(2):
                add_qk(0, jh, 2)
            for st in (8, 9, 10, 11):
                add_v(0, st)
            for jh in range(2):
                add_qk(0, jh, 3)
            for st in (12, 13, 14, 15):
                add_v(0, st)
            # entries needed before pair-0 chunk t: QK sc<=t//2, V st<=2t+1
            need0 = {2: 8, 3: 8, 4: 16, 5: 16, 6: 24, 7: 24}

            # entries of a pair's 32-entry filler block needed before its
            # chunk t: QK sc<=t//2 (4 entries each) + V st<=2t+1
            needN = {0: 8, 2: 14, 3: 16, 4: 22, 5: 24, 6: 30, 7: 32}
            pair_base = {}
            for p in range(NP):
                if p < NP - 1:
                    pair_base[p + 1] = added[0]
                    for sc in range(SC):
                        for jh in range(2):
                            add_qk(p + 1, jh, sc)
                        for st in range(4 * sc, 4 * sc + 4):
                            add_v(p + 1, st)
                for t in range(IC):
                    if p == 0 and t in need0:
                        pump_until(need0[t])
                    if p > 0 and t in needN:
                        pump_until(pair_base[p] + needN[t])
                    if p == NP - 1 and t >= 1:
                        st0 = 2 * (t - 1)
                        h1, h2 = {}, {}
                        addf(870, lambda st=st0, h=h1: out_half_a(st, h))
                        addf(900, lambda st=st0, h=h1: out_half_b(st, h))
                        addf(870, lambda st=st0 + 1, h=h2: out_half_a(st, h))
                        addf(900, lambda st=st0 + 1, h=h2: out_half_b(st, h))
                    attn_chunk(p, t, lambda: pump(620))
            flush_pend()
            pump_until(added[0])
            for st in range(2 * (IC - 1), ST):
                out_unit(st)
    nc.compile()
    return nc


def make_tables(S=2048, hd=64):
    inv_freq = 1.0 / (ROPE_BASE ** (np.arange(0, hd, 2, dtype=np.float64) / hd))
    t = np.arange(S, dtype=np.float64)
    freqs = np.outer(t, inv_freq)                    # [S, 32]
    emb = np.concatenate([freqs, freqs], axis=-1)    # [S, 64]
    cos1 = np.cos(emb).T.astype(np.float32)          # [64, S]
    sin1 = np.sin(emb).T.astype(np.float32)
    sign = np.array([-1.0 if r % 2 == 0 else 1.0 for r in range(64)])
    cos1 = cos1[PERM64]
    sin1 = sin1[PERM64] * sign[:, None]
    cosT = cos1.astype(np.float16)                   # [64, S] half table
    sinT = sin1.astype(np.float16)
    tri = np.tril(np.ones((128, 128), np.float32)).T.astype(np.float16)
    # rotP.T @ q = rotate_half(q) with the sign folded in, per 64-dim head
    rotP = np.zeros((128, 128), np.float16)
    for j in range(128):
        base = (j // 64) * 64
        jj = j % 64
        if jj < 32:
            rotP[base + jj + 32, j] = -1.0
        else:
            rotP[base + jj - 32, j] = 1.0
    return cosT, sinT, tri, rotP


def make_core_inputs(x, Wqkv, Wout, b, g, HL=8, hd=64):
    """Host-side shard prep for core (batch b, head group g)."""
    B, S, D = x.shape
    H = D // hd
    NP = HL // 2
    heads = list(range(g * HL, (g + 1) * HL))
    Wq = Wqkv[:, 0:D].reshape(D, H, hd)
    Wk = Wqkv[:, D:2 * D].reshape(D, H, hd)
    Wv = Wqkv[:, 2 * D:3 * D].reshape(D, H, hd)
    # Wqk column order: per pair p: q(h0),q(h1),k(h0),k(h1)
    blocks = []
    for p in range(NP):
        h0, h1 = heads[2 * p], heads[2 * p + 1]
        blocks.append(np.concatenate(
            [Wq[:, h0][:, PERM64], Wq[:, h1][:, PERM64],
             Wk[:, h0][:, PERM64], Wk[:, h1][:, PERM64]], axis=1))
    Wqk_full = np.concatenate(blocks, axis=1)               # [D, NP*256]
    Wv_full = Wv[:, heads].reshape(D, HL * hd)              # [D, 512]
    Wout_full = Wout[g * HL * hd:(g + 1) * HL * hd, :]      # [512, D]
    # fat-line reorders: partition dim first, contraction tiles on free dim
    Wqk_host = np.ascontiguousarray(
        Wqk_full.reshape(8, 128, NP * 256).transpose(1, 0, 2), np.float16)
    Wv_host = np.ascontiguousarray(
        Wv_full.reshape(8, 128, HL * hd).transpose(1, 0, 2), np.float16)
    Wout_host = np.ascontiguousarray(
        Wout_full.reshape(NP, 128, 2, 512).transpose(1, 0, 2, 3), np.float16)
    xT = x[b].T                                             # [D, S]
    x_host = np.ascontiguousarray(
        xT.reshape(8, 128, 4, 512).transpose(1, 2, 0, 3), np.float16)
    cosT, sinT, tri, rotP = make_tables(S, hd)
    return {"xh": x_host, "Wqk": Wqk_host, "Wv": Wv_host, "Wout": Wout_host,
            "cosT": cosT, "sinT": sinT, "tri": tri, "rotP": rotP}


_NC_CACHE = {}
TRACE = False          # test-only: capture NTFF profile + exec time
LAST_EXEC_NS = None
LAST_RESULT = None


def _enable_ntff_hook():
    import types
    import trn_agent_boot.trn_boot as tb
    import concourse.bass_utils as bu
    m = types.ModuleType("antenv.axon_hooks")
    _hook = [None]
    m.set_axon_ntff_profile_hook = lambda h: _hook.__setitem__(0, h)
    m.get_axon_ntff_profile_hook = lambda: _hook[0]
    sys.modules["antenv.axon_hooks"] = m
    m.set_axon_ntff_profile_hook(
        tb._ntff_profile_via_ctypes("/opt/axon/libaxon_pjrt.so"))
    bu.upload_artifacts = lambda tmpdir: ""


def kernel(x, Wqkv, Wout):
    global LAST_EXEC_NS, LAST_RESULT
    B, S, D = x.shape
    key = (B, S, D)
    if key not in _NC_CACHE:
        _NC_CACHE[key] = build_core_program(S=S, D=D)
    nc = _NC_CACHE[key]
    in_maps = []
    for core in range(8):
        b, g = core // 2, core % 2
        in_maps.append(make_core_inputs(np.asarray(x), np.asarray(Wqkv),
                                        np.asarray(Wout), b, g))
    kw = {}
    if TRACE:
        _enable_ntff_hook()
        kw = dict(trace=True, trace_cores=[0])
    res = run_bass_kernel_spmd(nc, in_maps, core_ids=list(range(8)), **kw)
    LAST_EXEC_NS = res.exec_time_ns
    LAST_RESULT = res
    y = np.empty((B, S, D), np.float32)
    for b in range(B):
        y[b] = (res.results[2 * b]["y"].astype(np.float32)
                + res.results[2 * b + 1]["y"].astype(np.float32))
    return y


# revision 36
# speedup vs baseline: 1.1801x; 1.1801x over previous
"""Causal self-attention with RoPE on 8 Trainium2 NeuronCores.

Sharding: batch (4) x head-group (2 groups of 8 heads) -> 8 cores.
Schedule: pair-level software pipeline.  The attention phase for head
pair p is ACT(exp)-bound (~1147ns/unit vs ~640ns PE/unit), so the PE
slack inside pair p's attention is filled with the V/QK projections +
RoPE of pair p+1, and with the output projection during the last pair.
Scores matmuls are emitted h0/h1-adjacent so the K=64 row-tiled pairs
run concurrently on disjoint PE row groups.  Inputs are host-reordered
so every DMA has fat (4-16KB) per-partition lines, split across three
engine queues; junk matmuls on tri warm the PE HAM clock during the
initial DMA latency.  Output is fp16 (summed per-batch on host).
"""
import sys

sys.path.insert(0, "/opt/trn_rl_repo")

import numpy as np

import concourse.bass as bass  # noqa: F401
import concourse.mybir as mybir
import concourse.tile as tile
from concourse import bacc
from concourse.bass_utils import run_bass_kernel_spmd

dt = mybir.dt
F32, F16 = dt.float32, dt.float16
ALU = mybir.AluOpType
EXP = mybir.ActivationFunctionType.Exp

ROPE_BASE = 10000.0
# swap adjacent partition pairs within each 32-partition quadrant
SHUF_MASK = [i ^ 1 for i in range(32)]
# row r of a 64-feature head holds feature PERM64[r] (pairs (k, k+32)
# interleaved so rotate_half becomes the adjacent-pair swap above)
PERM64 = [(r // 2) if r % 2 == 0 else (r // 2 + 32) for r in range(64)]


def build_core_program(S=2048, D=1024, HL=8, hd=64):
    assert hd == 64
    NP = HL // 2           # 4 head pairs
    DT = D // 128          # 8 contraction tiles
    SC = S // 512          # 4 sequence chunks
    ST = S // 128          # 16 seq tiles
    IC = S // 256          # 8 attention query chunks per pair
    NDC = D // 512         # 2 out-proj column chunks
    scale = hd ** -0.5

    nc = bacc.Bacc("TRN2", target_bir_lowering=False, debug=False)
    # host-reordered fat-line layouts
    x_d = nc.dram_tensor("xh", [128, SC, DT, 512], F16, kind="ExternalInput").ap()
    Wqk_d = nc.dram_tensor("Wqk", [128, DT, NP * 256], F16, kind="ExternalInput").ap()
    Wv_d = nc.dram_tensor("Wv", [128, DT, HL * hd], F16, kind="ExternalInput").ap()
    Wout_d = nc.dram_tensor("Wout", [128, NP, NDC, 512], F16, kind="ExternalInput").ap()
    cos_d = nc.dram_tensor("cosT", [64, S], F16, kind="ExternalInput").ap()
    sin_d = nc.dram_tensor("sinT", [64, S], F16, kind="ExternalInput").ap()
    tri_d = nc.dram_tensor("tri", [128, 128], F16, kind="ExternalInput").ap()
    rotP_d = nc.dram_tensor("rotP", [128, 128], F16, kind="ExternalInput").ap()
    y_d = nc.dram_tensor("y", [S, D], F16, kind="ExternalOutput").ap()

    with tile.TileContext(nc) as tc:
        with tc.tile_pool(name="persist", bufs=1) as pp, \
             tc.tile_pool(name="q16p", bufs=4) as q16p, \
             tc.tile_pool(name="expp", bufs=6) as expp, \
             tc.tile_pool(name="normp", bufs=4) as normp, \
             tc.tile_pool(name="ystp", bufs=2) as ystp, \
             tc.tile_pool(name="projps", bufs=2, space="PSUM") as projps, \
             tc.tile_pool(name="sps", bufs=2, space="PSUM") as sps, \
             tc.tile_pool(name="pavp", bufs=2, space="PSUM") as pavp:

            # ---------------- persistent SBUF tensors ----------------
            xq = pp.tile([128, SC, DT, 512], F16, tag="xq")
            wv = pp.tile([128, DT, HL * hd], F16, tag="wv")
            wqk = pp.tile([128, DT, NP * 256], F16, tag="wqk")
            wout = pp.tile([128, NP, NDC, 512], F16, tag="wout")
            qkT = [pp.tile([128, S], F16, tag=f"qkT{j}", name=f"qkT{j}")
                   for j in range(2 * NP)]
            v_sb = pp.tile([128, ST, HL, 66], F16, tag="v_sb")
            outT = [pp.tile([128, S], F16, tag=f"outT{p}", name=f"outT{p}")
                    for p in range(NP)]
            cosT = pp.tile([128, S], F16, tag="cosT")
            sinT = pp.tile([128, S], F16, tag="sinT")
            tri = pp.tile([128, 128], F16, tag="tri")
            rotP = pp.tile([128, 128], F16, tag="rotP")

            # DMAs: single proven sync queue; ordered by first consumer.
            nc.sync.dma_start(out=tri[:], in_=tri_d[:])
            nc.sync.dma_start(out=rotP[:], in_=rotP_d[:])
            nc.sync.dma_start(out=wv[:], in_=Wv_d[:])
            nc.sync.dma_start(out=xq[:, 0, :, :], in_=x_d[:, 0, :, :])
            nc.sync.dma_start(out=wqk[:], in_=Wqk_d[:])
            nc.sync.dma_start(out=cosT[0:64, :], in_=cos_d[:])
            nc.sync.dma_start(out=sinT[0:64, :], in_=sin_d[:])
            for sc in range(1, SC):
                nc.sync.dma_start(out=xq[:, sc, :, :], in_=x_d[:, sc, :, :])
            nc.sync.dma_start(out=wout[:], in_=Wout_d[:])
            nc.vector.memset(v_sb[:, :, :, 64:65], 1.0)

            # PE warmup on tri during initial DMA latency (HAM + P-state)
            junk = sps.tile([128, 2, 2, 256], F32, tag="spt", name="junk")
            for _ in range(40):
                nc.tensor.matmul(junk[:, 0, 0, 0:128], tri[:], tri[:],
                                 start=True, stop=True)


            # ---------------- filler units (PE work for pipelining) --------
            def v_unit(p, st):
                sc, r = st // 4, st % 4
                vps = projps.tile([128, 128], F32, tag="pjps", name="vps")
                for ddt in range(DT):
                    nc.tensor.matmul(
                        vps[:],
                        xq[:, sc, ddt, r * 128:(r + 1) * 128],
                        wv[:, ddt, p * 128:(p + 1) * 128],
                        start=(ddt == 0), stop=(ddt == DT - 1))
                nc.vector.tensor_copy(
                    v_sb[:, st, 2 * p:2 * p + 2, 0:64],
                    vps[:].rearrange("p (h c) -> p h c", h=2))

            def qk_half_a(p, jh, sc, st8):
                qkps = projps.tile([128, 512], F32, tag="pjps", name="qkps")
                st8["qkps"] = qkps
                for ddt in range(DT // 2):
                    nc.tensor.matmul(
                        qkps[:],
                        wqk[:, ddt, p * 256 + jh * 128:p * 256 + jh * 128 + 128],
                        xq[:, sc, ddt, :],
                        start=(ddt == 0), stop=False)

            def qk_half_b(p, jh, sc, st8):
                jt = 2 * p + jh
                ss = slice(sc * 512, (sc + 1) * 512)
                qkps = st8["qkps"]
                for ddt in range(DT // 2, DT):
                    nc.tensor.matmul(
                        qkps[:],
                        wqk[:, ddt, p * 256 + jh * 128:p * 256 + jh * 128 + 128],
                        xq[:, sc, ddt, :],
                        start=False, stop=(ddt == DT - 1))
                q16 = q16p.tile([128, 512], F16, tag="q16", name="q16")
                nc.vector.tensor_copy(q16[:], qkps[:])
                rotm = q16p.tile([128, 512], F16, tag="q16", name="rotm")
                nc.vector.stream_shuffle(rotm[:], q16[:], SHUF_MASK)
                nc.vector.tensor_tensor(rotm[:], rotm[:], sinT[:, ss],
                                        ALU.mult)
                nc.vector.tensor_tensor(qkT[jt][:, ss], q16[:], cosT[:, ss],
                                        ALU.mult)
                nc.vector.tensor_tensor(qkT[jt][:, ss], qkT[jt][:, ss],
                                        rotm[:], ALU.add)

            def qk_unit(p, jh, sc):
                st8 = {}
                qk_half_a(p, jh, sc, st8)
                qk_half_b(p, jh, sc, st8)

            def out_half_a(st, h8):
                yp2 = [projps.tile([128, 512], F32, tag="pjps", name="yps")
                       for _ in range(NDC)]
                h8["yp2"] = yp2
                for pb in range(NP // 2):
                    for dc in range(NDC):
                        nc.tensor.matmul(
                            yp2[dc][:],
                            outT[pb][:, st * 128:(st + 1) * 128],
                            wout[:, pb, dc, :],
                            start=(pb == 0), stop=False)

            def out_half_b(st, h8):
                yp2 = h8["yp2"]
                for pb in range(NP // 2, NP):
                    for dc in range(NDC):
                        nc.tensor.matmul(
                            yp2[dc][:],
                            outT[pb][:, st * 128:(st + 1) * 128],
                            wout[:, pb, dc, :],
                            start=False, stop=(pb == NP - 1))
                yst = ystp.tile([128, NDC * 512], F16, tag="yst", name="yst")
                for dc in range(NDC):
                    nc.vector.tensor_copy(
                        yst[:, dc * 512:(dc + 1) * 512], yp2[dc][:])
                nc.sync.dma_start(
                    out=y_d[st * 128:(st + 1) * 128, :], in_=yst[:])

            def out_unit(st):
                h8 = {}
                out_half_a(st, h8)
                out_half_b(st, h8)

            # ---------------- attention, software-pipelined ----------------
            pend = {"av": None, "norm": None}

            def flush_pend():
                if pend["av"] is not None:
                    pend["av"]()
                    pend["av"] = None
                if pend["norm"] is not None:
                    pend["norm"]()
                    pend["norm"] = None

            def attn_chunk(p, t, unit_cb):
                # unit_cb() is invoked once per key-unit, right after that
                # unit's PE attention work, to pump filler matmuls into the
                # PE stream while ACT chews on exp.
                qT, kT = qkT[2 * p], qkT[2 * p + 1]
                qs = slice(t * 256, (t + 1) * 256)
                pav = []
                njp = t + 1
                prev = None   # (jp, et) awaiting AV emission

                def emit_av(jp, et):
                    for jj in range(2):
                        jt = 2 * jp + jj
                        qlo = 128 if (jp == njp - 1 and jj == 1) else 0
                        for hh in range(2):
                            h = 2 * p + hh
                            nc.tensor.matmul(
                                pav[hh][0:65, qlo:256],
                                v_sb[:, jt, h, 0:65],
                                et[:, hh, jj, qlo:256],
                                start=(jp == 0 and jj == 0),
                                stop=(jp == njp - 1 and jj == 1))

                for jp in range(njp):
                    spt = sps.tile([128, 2, 2, 256], F32, tag="spt",
                                   name="spt")
                    diag = (jp == njp - 1)
                    for jj in range(2):
                        jt = 2 * jp + jj
                        for hh in range(2):
                            hb = 64 * hh
                            nc.tensor.matmul(
                                spt[:, hh, jj, :],
                                kT[hb:hb + 64, jt * 128:(jt + 1) * 128],
                                qT[hb:hb + 64, qs],
                                start=True, stop=True)
                    et = expp.tile([128, 2, 2, 256], F16, tag="expp",
                                   name="et")
                    nc.scalar.activation(et[:], spt[:], EXP, scale=scale)
                    if diag:
                        for hh in range(2):
                            for jj in range(2):
                                nc.vector.tensor_tensor(
                                    et[:, hh, jj, 128 * jj:128 * jj + 128],
                                    et[:, hh, jj, 128 * jj:128 * jj + 128],
                                    tri[:], ALU.mult)
                    if jp == 0:
                        flush_pend()
                        pav.extend(
                            pavp.tile([128, 256], F32, tag="pav", name="pav")
                            for _ in range(2))
                    else:
                        emit_av(*prev)
                    unit_cb()
                    prev = (jp, et)

                jp_l, et_l = prev
                pend["av"] = lambda: emit_av(jp_l, et_l)

                def emit_norm(pav=pav, p=p, qs=qs):
                    for hh in range(2):
                        srow = normp.tile([1, 256], F32, tag="srow",
                                          name="srow")
                        nc.vector.tensor_copy(srow[:], pav[hh][64:65, :])
                        rstage = normp.tile([1, 256], F32, tag="rst",
                                            name="rst")
                        nc.vector.reciprocal_approx_fast(
                            out=rstage[:], in_=srow[:])
                        brec = normp.tile([64, 256], F32, tag="brec",
                                          name="brec")
                        nc.gpsimd.partition_broadcast(brec[:], rstage[:])
                        nc.vector.tensor_tensor(
                            outT[p][64 * hh:64 * hh + 64, qs],
                            pav[hh][0:64, :], brec[:], ALU.mult)

                pend["norm"] = emit_norm

            # ---------------- schedule ----------------
            # Filler FIFO pumped one credit-slice per attention unit; gated
            # force-drains guarantee a pair's inputs exist before use.
            fill = []
            emitted = [0]
            added = [0]

            def pump(credit):
                while fill and credit > 0:
                    cost, fn = fill.pop(0)
                    fn()
                    emitted[0] += 1
                    credit -= cost

            def pump_until(n):
                while emitted[0] < n:
                    cost, fn = fill.pop(0)
                    fn()
                    emitted[0] += 1

            def addf(cost, fn):
                fill.append((cost, fn))
                added[0] += 1

            def add_v(p, st):
                addf(430, lambda: v_unit(p, st))

            def add_qk(p, jh, sc):
                st8 = {}
                addf(870, lambda: qk_half_a(p, jh, sc, st8))
                addf(870, lambda: qk_half_b(p, jh, sc, st8))

            # Prologue: just enough of pair 0 for its first two chunks.
            for st in range(4):
                v_unit(0, st)
            nc.vector.tensor_copy(cosT[64:128, :], cosT[0:64, :])
            nc.vector.tensor_copy(sinT[64:128, :], sinT[0:64, :])
            for jh in range(2):
                qk_unit(0, jh, 0)
            # Pair-0 leftovers, ordered so chunk gates pass; then pair 1.
            for jh in range(2):
                add_qk(0, jh, 1)
            for st in (4, 5, 6, 7):
                add_v(0, st)
            for jh in range(2):
                add_qk(0, jh, 2)
            for st in (8, 9, 10, 11):
                add_v(0, st)
            for jh in range(2):
                add_qk(0, jh, 3)
            for st in (12, 13, 14, 15):
                add_v(0, st)
            # entries needed before pair-0 chunk t: QK sc<=t//2, V st<=2t+1
            need0 = {2: 8, 3: 8, 4: 16, 5: 16, 6: 24, 7: 24}

            # entries of a pair's 32-entry filler block needed before its
            # chunk t: QK sc<=t//2 (4 entries each) + V st<=2t+1
            needN = {0: 8, 2: 14, 3: 16, 4: 22, 5: 24, 6: 30, 7: 32}
            pair_base = {}
            for p in range(NP):
                if p < NP - 1:
                    pair_base[p + 1] = added[0]
                    for sc in range(SC):
                        for jh in range(2):
                            add_qk(p + 1, jh, sc)
                        for st in range(4 * sc, 4 * sc + 4):
                            add_v(p + 1, st)
                for t in range(IC):
                    if p == 0 and t in need0:
                        pump_until(need0[t])
                    if p > 0 and t in needN:
                        pump_until(pair_base[p] + needN[t])
                    if p == NP - 1 and t >= 1:
                        st0 = 2 * (t - 1)
                        h1, h2 = {}, {}
                        addf(870, lambda st=st0, h=h1: out_half_a(st, h))
                        addf(900, lambda st=st0, h=h1: out_half_b(st, h))
                        addf(870, lambda st=st0 + 1, h=h2: out_half_a(st, h))
                        addf(900, lambda st=st0 + 1, h=h2: out_half_b(st, h))
                    attn_chunk(p, t, lambda: pump(620))
            flush_pend()
            pump_until(added[0])
            for st in range(2 * (IC - 1), ST):
                out_unit(st)
    nc.compile()
    return nc


def make_tables(S=2048, hd=64):
    inv_freq = 1.0 / (ROPE_BASE ** (np.arange(0, hd, 2, dtype=np.float64) / hd))
    t = np.arange(S, dtype=np.float64)
    freqs = np.outer(t, inv_freq)                    # [S, 32]
    emb = np.concatenate([freqs, freqs], axis=-1)    # [S, 64]
    cos1 = np.cos(emb).T.astype(np.float32)          # [64, S]
    sin1 = np.sin(emb).T.astype(np.float32)
    sign = np.array([-1.0 if r % 2 == 0 else 1.0 for r in range(64)])
    cos1 = cos1[PERM64]
    sin1 = sin1[PERM64] * sign[:, None]
    cosT = cos1.astype(np.float16)                   # [64, S] half table
    sinT = sin1.astype(np.float16)
    tri = np.tril(np.ones((128, 128), np.float32)).T.astype(np.float16)
    # rotP.T @ q = rotate_half(q) with the sign folded in, per 64-dim head
    rotP = np.zeros((128, 128), np.float16)
    for j in range(128):
        base = (j // 64) * 64
        jj = j % 64
        if jj < 32:
            rotP[base + jj + 32, j] = -1.0
        else:
            rotP[base + jj - 32, j] = 1.0
    return cosT, sinT, tri, rotP


def make_core_inputs(x, Wqkv, Wout, b, g, HL=8, hd=64):
    """Host-side shard prep for core (batch b, head group g)."""
    B, S, D = x.shape
    H = D // hd
    NP = HL // 2
    heads = list(range(g * HL, (g + 1) * HL))
    Wq = Wqkv[:, 0:D].reshape(D, H, hd)
    Wk = Wqkv[:, D:2 * D].reshape(D, H, hd)
    Wv = Wqkv[:, 2 * D:3 * D].reshape(D, H, hd)
    # Wqk column order: per pair p: q(h0),q(h1),k(h0),k(h1)
    blocks = []
    for p in range(NP):
        h0, h1 = heads[2 * p], heads[2 * p + 1]
        blocks.append(np.concatenate(
            [Wq[:, h0][:, PERM64], Wq[:, h1][:, PERM64],
             Wk[:, h0][:, PERM64], Wk[:, h1][:, PERM64]], axis=1))
    Wqk_full = np.concatenate(blocks, axis=1)               # [D, NP*256]
    Wv_full = Wv[:, heads].reshape(D, HL * hd)              # [D, 512]
    Wout_full = Wout[g * HL * hd:(g + 1) * HL * hd, :]      # [512, D]
    # fat-line reorders: partition dim first, contraction tiles on free dim
    Wqk_host = np.ascontiguousarray(
        Wqk_full.reshape(8, 128, NP * 256).transpose(1, 0, 2), np.float16)
    Wv_host = np.ascontiguousarray(
        Wv_full.reshape(8, 128, HL * hd).transpose(1, 0, 2), np.float16)
    Wout_host = np.ascontiguousarray(
        Wout_full.reshape(NP, 128, 2, 512).transpose(1, 0, 2, 3), np.float16)
    xT = x[b].T                                             # [D, S]
    x_host = np.ascontiguousarray(
        xT.reshape(8, 128, 4, 512).transpose(1, 2, 0, 3), np.float16)
    cosT, sinT, tri, rotP = make_tables(S, hd)
    return {"xh": x_host, "Wqk": Wqk_host, "Wv": Wv_host, "Wout": Wout_host,
            "cosT": cosT, "sinT": sinT, "tri": tri, "rotP": rotP}


_NC_CACHE = {}
TRACE = False          # test-only: capture NTFF profile + exec time
LAST_EXEC_NS = None
LAST_RESULT = None


def _enable_ntff_hook():
    import types
    import trn_agent_boot.trn_boot as tb
    import concourse.bass_utils as bu
    m = types.ModuleType("antenv.axon_hooks")
    _hook = [None]
    m.set_axon_ntff_profile_hook = lambda h: _hook.__setitem__(0, h)
    m.get_axon_ntff_profile_hook = lambda: _hook[0]
    sys.modules["antenv.axon_hooks"] = m
    m.set_axon_ntff_profile_hook(
        tb._ntff_profile_via_ctypes("/opt/axon/libaxon_pjrt.so"))
    bu.upload_artifacts = lambda tmpdir: ""


def kernel(x, Wqkv, Wout):
    global LAST_EXEC_NS, LAST_RESULT
    B, S, D = x.shape
    key = (B, S, D)
    if key not in _NC_CACHE:
        _NC_CACHE[key] = build_core_program(S=S, D=D)
    nc = _NC_CACHE[key]
    in_maps = []
    for core in range(8):
        b, g = core // 2, core % 2
        in_maps.append(make_core_inputs(np.asarray(x), np.asarray(Wqkv),
                                        np.asarray(Wout), b, g))
    kw = {}
    if TRACE:
        _enable_ntff_hook()
        kw = dict(trace=True, trace_cores=[0])
    res = run_bass_kernel_spmd(nc, in_maps, core_ids=list(range(8)), **kw)
    LAST_EXEC_NS = res.exec_time_ns
    LAST_RESULT = res
    y = np.empty((B, S, D), np.float32)
    for b in range(B):
        y[b] = (res.results[2 * b]["y"].astype(np.float32)
                + res.results[2 * b + 1]["y"].astype(np.float32))
    return y
